# revision 5
# baseline (speedup 1.0000x reference)
"""AdjacentAttention on 8 TRN2 NeuronCores.

Strategy (all shapes hardcoded for B=1, N=10000, A=32, D=256, H=4, DH=64):

Host:
  - kv projection commutes with the neighbor gather, so the device computes a
    kv table (x @ Wkv, bf16) once and gathers *projected* rows, 32x less
    matmul work than the reference's gather-then-project.
  - ~50% of neighbors are masked out.  The host compacts each node's
    neighbor list to its valid entries, sorts nodes by degree, and deals
    them into 10 degree-homogeneous tile groups of 1024 (128 nodes x 8
    cores), so each tile only gathers/computes its group-max degree a_t
    instead of A=32.
  - x is passed pre-transposed (and bf16) so no on-device transposes are
    needed for the matmuls; attention-scale is folded into Wq.

Device (SPMD, identical program per core, no collectives):
  Phase A: kv table = x @ Wkv for all 10112 padded rows -> HBM scratch (bf16).
  Phase B: q tiles for this core's 1280 permuted nodes.
  Phase C: per node-tile: dma_gather of kv rows for (node, valid-neighbor)
    pairs + a null slot; q.k on DVE (bf16 2x) + halving trees; softmax with
    f32 denominators; attn-weighted v-sum; PE transpose + out-projection.
"""

import os

import numpy as np
import ml_dtypes

import bass_rust
import concourse.bacc as bacc
import concourse.tile as tile
from concourse import mybir
from concourse.bass_utils import run_bass_kernel_spmd

BF = ml_dtypes.bfloat16

N, A, D, H, DH = 10000, 32, 256, 4, 64
NCORES, P, NT = 8, 128, 10
GROUP = NCORES * P            # 1024 nodes per tile-group
NPAD = NT * GROUP             # 10240
KV_TILES = (N + P - 1) // P   # 79
NKV = KV_TILES * P            # 10112 padded kv-table rows
HD = H * DH                   # 256
KVW = 2 * HD                  # 512 (k|v row width)

LAST_EXEC_NS = None

_MULT = None
_ADD = None


def _build(a_ts):
    stage = int(os.environ.get("KERNEL_STAGE", "4"))
    nc = bacc.Bacc("TRN2", target_bir_lowering=False)
    bf = mybir.dt.bfloat16
    f32 = mybir.dt.float32
    mult = mybir.AluOpType.mult
    add = mybir.AluOpType.add

    aas = [a + 1 for a in a_ts]
    idxcols = 8 * sum(a_ts)
    mcols = sum(aas)

    xT = nc.declare_dram_parameter("xT", [P, 2, NKV], bf, isOutput=False)
    xpT = nc.declare_dram_parameter("xpT", [P, 2, NT * P], bf, isOutput=False)
    wq = nc.declare_dram_parameter("wq", [P, 2, HD], bf, isOutput=False)
    wkv = nc.declare_dram_parameter("wkv", [P, 2, KVW], bf, isOutput=False)
    wo = nc.declare_dram_parameter("wo", [P, 2, D], bf, isOutput=False)
    bo_p = nc.declare_dram_parameter("bo", [1, D], bf, isOutput=False)
    nullkv = nc.declare_dram_parameter("nullkv", [P, KVW], bf, isOutput=False)
    ident_p = nc.declare_dram_parameter("ident", [P, P], bf, isOutput=False)
    idxs_p = nc.declare_dram_parameter("idxs", [P, idxcols], mybir.dt.int16, isOutput=False)
    masks_p = nc.declare_dram_parameter("masks", [P, mcols], f32, isOutput=False)
    out_p = nc.declare_dram_parameter("out", [NT * P, D], f32, isOutput=True)

    kv_dram = nc.dram_tensor("kv_scratch", [NKV, KVW], bf)

    with tile.TileContext(nc) as tc:
        with (
            tc.tile_pool(name="big", bufs=1) as big,
            tc.tile_pool(name="singles", bufs=1) as singles,
            tc.tile_pool(name="kvstage", bufs=4) as kvstage,
            tc.tile_pool(name="work", bufs=2) as work,
            tc.tile_pool(name="small", bufs=3) as small,
            tc.tile_pool(name="psA", bufs=2, space="PSUM") as psA,
            tc.tile_pool(name="psT", bufs=2, space="PSUM") as psT,
            tc.tile_pool(name="psF", bufs=2, space="PSUM") as psF,
        ):
            # ---------- constants ----------
            wq_sb = singles.tile([P, 2, HD], bf)
            nc.sync.dma_start(out=wq_sb[:], in_=wq[:])
            wkv_sb = singles.tile([P, 2, KVW], bf)
            nc.sync.dma_start(out=wkv_sb[:], in_=wkv[:])
            wo_sb = singles.tile([P, 2, D], bf)
            nc.sync.dma_start(out=wo_sb[:], in_=wo[:])
            bo_sb = singles.tile([1, D], bf)
            nc.sync.dma_start(out=bo_sb[:], in_=bo_p[:])
            nullkv_sb = singles.tile([P, KVW], bf)
            nc.sync.dma_start(out=nullkv_sb[:], in_=nullkv[:])
            ident_sb = singles.tile([P, P], bf)
            nc.sync.dma_start(out=ident_sb[:], in_=ident_p[:])
            idx_sb = singles.tile([P, idxcols], mybir.dt.int16)
            idx_dma = nc.sync.dma_start(out=idx_sb[:], in_=idxs_p[:])
            mask_sb = singles.tile([P, mcols], f32)
            nc.sync.dma_start(out=mask_sb[:], in_=masks_p[:])
            ones1 = singles.tile([1, P], bf)
            nc.vector.memset(ones1[:], 1.0)

            # ---------- phase A: kv table ----------
            x_sb = big.tile([P, 2, NKV], bf)
            nc.sync.dma_start(out=x_sb[:], in_=xT[:])
            kv_writes = []
            for i in range(KV_TILES):
                ps = psA.tile([P, KVW], f32, space="PSUM", tag="psA")
                nc.tensor.matmul(
                    out=ps[:], lhsT=x_sb[:, 0, i * P:(i + 1) * P],
                    rhs=wkv_sb[:, 0, :], start=True, stop=False)
                nc.tensor.matmul(
                    out=ps[:], lhsT=x_sb[:, 1, i * P:(i + 1) * P],
                    rhs=wkv_sb[:, 1, :], start=False, stop=True)
                st = kvstage.tile([P, KVW], bf, tag="kvstage")
                nc.scalar.copy(out=st[:], in_=ps[:])
                kv_writes.append(
                    nc.sync.dma_start(out=kv_dram[i * P:(i + 1) * P, :], in_=st[:]))

            # ---------- phase B: q tiles ----------
            xp_sb = singles.tile([P, 2, NT * P], bf)
            nc.sync.dma_start(out=xp_sb[:], in_=xpT[:])
            q_sb = singles.tile([P, NT, HD], bf)
            for t in range(NT):
                psq = psF.tile([P, HD], f32, space="PSUM", tag="psF")
                nc.tensor.matmul(
                    out=psq[:], lhsT=xp_sb[:, 0, t * P:(t + 1) * P],
                    rhs=wq_sb[:, 0, :], start=True, stop=False)
                nc.tensor.matmul(
                    out=psq[:], lhsT=xp_sb[:, 1, t * P:(t + 1) * P],
                    rhs=wq_sb[:, 1, :], start=False, stop=True)
                nc.scalar.copy(out=q_sb[:, t, :], in_=psq[:])

            # ---------- phase C: attention per tile ----------
            if stage < 2:
                a_ts_eff = []
            else:
                a_ts_eff = a_ts
            io = 0
            mo = 0
            for t in range(len(a_ts_eff)):
                a = a_ts[t]
                aa = a + 1
                kv_g = work.tile([P, aa, KVW], bf, tag="kvg")
                nc.scalar.copy(out=kv_g[:, 0, :], in_=nullkv_sb[:])
                gi = nc.gpsimd.dma_gather(
                    kv_g[:, 1:, :], kv_dram[:], idx_sb[:, io:io + 8 * a],
                    num_idxs=P * a, num_idxs_reg=P * a, elem_size=KVW,
                    single_packet=False)
                # Tile's auto-dep tracking misses dma_gather's *input* APs
                # (idx tile + DRAM source); add the edges explicitly.
                bass_rust.add_dep_helper(gi.ins, idx_dma.ins,
                                         reason="gather reads idx blob")
                for kw in kv_writes:
                    bass_rust.add_dep_helper(gi.ins, kw.ins,
                                             reason="gather reads kv table")

                if stage < 3:
                    io += 8 * a
                    mo += aa
                    continue
                k4 = kv_g[:, :, 0:HD].rearrange("p a (h d) -> p a h d", d=DH)
                qb = (q_sb[:, t:t + 1, :]
                      .rearrange("p o (h d) -> p o h d", d=DH)
                      .broadcast_to([P, aa, H, DH]))
                nc.vector.tensor_tensor(out=k4, in0=k4, in1=qb, op=mult)
                w = DH
                while w > 1:
                    h2 = w // 2
                    nc.vector.tensor_tensor(
                        out=k4[:, :, :, 0:h2], in0=k4[:, :, :, 0:h2],
                        in1=k4[:, :, :, h2:w], op=add)
                    w = h2

                sim = kv_g[:, :, 0:HD:DH]          # [P, aa, H] strided
                exp_s = small.tile([P, aa, H], f32, tag="exp")
                nc.scalar.activation(
                    out=exp_s[:], in_=sim,
                    func=mybir.ActivationFunctionType.Exp)
                mb = (mask_sb[:, mo:mo + aa]
                      .rearrange("p (a o) -> p a o", o=1)
                      .broadcast_to([P, aa, H]))
                nc.vector.tensor_tensor(out=exp_s[:], in0=exp_s[:], in1=mb, op=mult)
                denom = small.tile([P, H], f32, tag="denom")
                nc.vector.tensor_reduce(
                    out=denom[:], in_=exp_s[:].rearrange("p a h -> p h a"),
                    axis=mybir.AxisListType.X, op=add)
                recip = small.tile([P, H], f32, tag="recip")
                nc.vector.reciprocal(out=recip[:], in_=denom[:])
                attn_b = small.tile([P, aa, H], bf, tag="attn")
                rb = (recip[:].rearrange("p (o h) -> p o h", o=1)
                      .broadcast_to([P, aa, H]))
                nc.vector.tensor_tensor(out=attn_b[:], in0=exp_s[:], in1=rb, op=mult)

                v4 = kv_g[:, :, HD:KVW].rearrange("p a (h d) -> p a h d", d=DH)
                ab = (attn_b[:].rearrange("p a (h o) -> p a h o", o=1)
                      .broadcast_to([P, aa, H, DH]))
                nc.vector.tensor_tensor(out=v4, in0=v4, in1=ab, op=mult)
                w = aa
                while w > 1:
                    h2 = w // 2
                    nc.vector.tensor_tensor(
                        out=v4[:, 0:h2], in0=v4[:, 0:h2],
                        in1=v4[:, h2:2 * h2], op=add)
                    if w % 2 == 1:
                        nc.vector.tensor_tensor(
                            out=v4[:, 0:1], in0=v4[:, 0:1],
                            in1=v4[:, w - 1:w], op=add)
                    w = h2

                if stage < 4:
                    io += 8 * a
                    mo += aa
                    continue
                out_attn = kv_g[:, 0, HD:KVW]      # [P, 256] bf16
                outT = work.tile([P, 2, P], bf, tag="outT")
                for j in range(2):
                    pst = psT.tile([P, P], bf, space="PSUM", tag="psT")
                    nc.tensor.transpose(
                        out=pst[:], in_=out_attn[:, j * P:(j + 1) * P],
                        identity=ident_sb[:])
                    nc.scalar.copy(out=outT[:, j, :], in_=pst[:])

                psf = psF.tile([P, D], f32, space="PSUM", tag="psF")
                nc.tensor.matmul(out=psf[:], lhsT=ones1[0:1, :], rhs=bo_sb[0:1, :],
                                 start=True, stop=False)
                nc.tensor.matmul(out=psf[:], lhsT=outT[:, 0, :], rhs=wo_sb[:, 0, :],
                                 start=False, stop=False)
                nc.tensor.matmul(out=psf[:], lhsT=outT[:, 1, :], rhs=wo_sb[:, 1, :],
                                 start=False, stop=True)
                outf = small.tile([P, D], f32, tag="outf")
                nc.scalar.copy(out=outf[:], in_=psf[:])
                nc.sync.dma_start(out=out_p[t * P:(t + 1) * P, :], in_=outf[:])

                io += 8 * a
                mo += aa

    nc.finalize()
    return nc


def _prep(x, adj, msk, Wq, Wkv, Wo, bo, null_k, null_v):
    """All host-side numpy prep. Returns (a_ts, in_maps, order)."""
    deg = msk.sum(1).astype(np.int64)
    order = np.concatenate([
        np.full(NPAD - N, -1, dtype=np.int64),
        np.argsort(deg, kind="stable"),
    ])

    a_ts = []
    for t in range(NT):
        grp = order[t * GROUP:(t + 1) * GROUP]
        real = grp[grp >= 0]
        mx = int(deg[real].max()) if real.size else 0
        a_ts.append(max(mx, 1))

    # compact each node's neighbor list: valid entries first
    sortcols = np.argsort(~msk, axis=1, kind="stable")
    comp = np.take_along_axis(adj, sortcols, axis=1).astype(np.int16)

    scale = DH ** -0.5
    xpad = np.zeros((NKV, D), np.float32)
    xpad[:N] = x
    xT_h = np.ascontiguousarray(
        xpad.T.reshape(2, P, NKV).transpose(1, 0, 2)).astype(BF)
    wq_h = np.ascontiguousarray(
        (Wq * scale).reshape(2, P, HD).transpose(1, 0, 2)).astype(BF)
    wkv_h = np.ascontiguousarray(
        Wkv.reshape(2, P, KVW).transpose(1, 0, 2)).astype(BF)
    wo_h = np.ascontiguousarray(
        Wo.reshape(2, P, D).transpose(1, 0, 2)).astype(BF)
    bo_h = bo.reshape(1, D).astype(BF)
    nullkv_h = np.tile(
        np.concatenate([null_k.reshape(-1), null_v.reshape(-1)]).reshape(1, KVW),
        (P, 1)).astype(BF)
    ident_h = np.eye(P, dtype=np.float32).astype(BF)

    in_maps = []
    for c in range(NCORES):
        xp = np.zeros((NT * P, D), np.float32)
        flats = []
        mblocks = []
        for t in range(NT):
            a = a_ts[t]
            nodes = order[t * GROUP + c * P: t * GROUP + (c + 1) * P]
            nn = np.maximum(nodes, 0)
            xp[t * P:(t + 1) * P][nodes >= 0] = x[nodes[nodes >= 0]]
            valid = (np.arange(a)[None, :] < deg[nn][:, None]) & (nodes >= 0)[:, None]
            blk = np.where(valid, comp[nn, :a], 0).astype(np.int16)  # [128, a]
            flats.append(blk.T.reshape(-1))                          # i = col*128+p
            m = np.zeros((P, 1 + a), np.float32)
            m[:, 0] = 1.0
            m[:, 1:] = valid
            mblocks.append(m)
        flat = np.concatenate(flats)
        idx_h = np.ascontiguousarray(
            np.tile(flat.reshape(-1, 16).T, (8, 1))).astype(np.int16)
        mask_h = np.ascontiguousarray(np.concatenate(mblocks, axis=1))
        xpT_h = np.ascontiguousarray(
            xp.T.reshape(2, P, NT * P).transpose(1, 0, 2)).astype(BF)
        in_maps.append({
            "xT": xT_h, "xpT": xpT_h, "wq": wq_h, "wkv": wkv_h, "wo": wo_h,
            "bo": bo_h, "nullkv": nullkv_h, "ident": ident_h,
            "idxs": idx_h, "masks": mask_h,
        })
    return a_ts, in_maps, order


def kernel(x, adj_kv_indices, mask, Wq, Wkv, Wo, bo, null_k, null_v):
    global LAST_EXEC_NS
    x = np.asarray(x, dtype=np.float32)[0]
    adj = np.asarray(adj_kv_indices)[0].astype(np.int64)
    msk = np.asarray(mask)[0].astype(bool)
    Wq = np.asarray(Wq, np.float32)
    Wkv = np.asarray(Wkv, np.float32)
    Wo = np.asarray(Wo, np.float32)
    bo = np.asarray(bo, np.float32)
    null_k = np.asarray(null_k, np.float32)
    null_v = np.asarray(null_v, np.float32)

    a_ts, in_maps, order = _prep(x, adj, msk, Wq, Wkv, Wo, bo, null_k, null_v)
    nc = _build(tuple(a_ts))
    res = run_bass_kernel_spmd(
        nc, in_maps, core_ids=list(range(NCORES)),
        trace=bool(os.environ.get("KERNEL_TRACE")))
    LAST_EXEC_NS = res.exec_time_ns

    out_full = np.zeros((N, D), np.float32)
    for c in range(NCORES):
        o = np.asarray(res.results[c]["out"])
        for t in range(NT):
            nodes = order[t * GROUP + c * P: t * GROUP + (c + 1) * P]
            sel = nodes >= 0
            out_full[nodes[sel]] = o[t * P:(t + 1) * P][sel]
    return out_full.reshape(1, N, D)


# revision 6
# speedup vs baseline: 1.2492x; 1.2492x over previous
"""AdjacentAttention on 8 TRN2 NeuronCores.

Strategy (all shapes hardcoded for B=1, N=10000, A=32, D=256, H=4, DH=64):

Host:
  - kv projection commutes with the neighbor gather, so the device computes a
    kv table (x @ Wkv, bf16) once and gathers *projected* rows, 32x less
    matmul work than the reference's gather-then-project.
  - ~50% of neighbors are masked out.  The host compacts each node's
    neighbor list to its valid entries, sorts nodes by degree, and deals
    them into 10 degree-homogeneous tile groups of 1024 (128 nodes x 8
    cores), so each tile only gathers/computes its group-max degree a_t
    instead of A=32.
  - x is passed pre-transposed (and bf16) so no on-device transposes are
    needed for the matmuls; attention-scale is folded into Wq.

Device (SPMD, identical program per core, no collectives):
  Phase A: kv table = x @ Wkv for all 10112 padded rows -> HBM scratch (bf16).
  Phase B: q tiles for this core's 1280 permuted nodes.
  Phase C: per node-tile: dma_gather of kv rows for (node, valid-neighbor)
    pairs + a null slot; q.k on DVE (bf16 2x) + halving trees; softmax with
    f32 denominators; attn-weighted v-sum; PE transpose + out-projection.
"""

import os

import numpy as np
import ml_dtypes

import bass_rust
import concourse.bacc as bacc
import concourse.tile as tile
from concourse import mybir
from concourse.bass_utils import run_bass_kernel_spmd

BF = ml_dtypes.bfloat16

N, A, D, H, DH = 10000, 32, 256, 4, 64
NCORES, P, NT = 8, 128, 10
GROUP = NCORES * P            # 1024 nodes per tile-group
NPAD = NT * GROUP             # 10240
KV_TILES = (N + P - 1) // P   # 79
NKV = KV_TILES * P            # 10112 padded kv-table rows
HD = H * DH                   # 256
KVW = 2 * HD                  # 512 (k|v row width)

LAST_EXEC_NS = None

_MULT = None
_ADD = None


def _build(a_ts):
    stage = int(os.environ.get("KERNEL_STAGE", "4"))
    nc = bacc.Bacc("TRN2", target_bir_lowering=False, num_swdge_queues=2)
    bf = mybir.dt.bfloat16
    f32 = mybir.dt.float32
    mult = mybir.AluOpType.mult
    add = mybir.AluOpType.add

    aas = [a + 1 for a in a_ts]
    idxcols = 8 * sum(a_ts)
    mcols = sum(aas)

    xT = nc.declare_dram_parameter("xT", [P, 2, NKV], bf, isOutput=False)
    xpT = nc.declare_dram_parameter("xpT", [P, 2, NT * P], bf, isOutput=False)
    wq = nc.declare_dram_parameter("wq", [P, 2, HD], bf, isOutput=False)
    wkv = nc.declare_dram_parameter("wkv", [P, 2, KVW], bf, isOutput=False)
    wo = nc.declare_dram_parameter("wo", [P, 2, D], bf, isOutput=False)
    bo_p = nc.declare_dram_parameter("bo", [1, D], bf, isOutput=False)
    nullkv = nc.declare_dram_parameter("nullkv", [P, KVW], bf, isOutput=False)
    ident_p = nc.declare_dram_parameter("ident", [P, P], bf, isOutput=False)
    idxs_p = nc.declare_dram_parameter("idxs", [P, idxcols], mybir.dt.int16, isOutput=False)
    masks_p = nc.declare_dram_parameter("masks", [P, mcols], f32, isOutput=False)
    out_p = nc.declare_dram_parameter("out", [NT * P, D], f32, isOutput=True)

    kv_dram = nc.dram_tensor("kv_scratch", [NKV, KVW], bf)

    with tile.TileContext(nc) as tc:
        with (
            tc.tile_pool(name="big", bufs=1) as big,
            tc.tile_pool(name="singles", bufs=1) as singles,
            tc.tile_pool(name="kvstage", bufs=4) as kvstage,
            tc.tile_pool(name="work", bufs=2) as work,
            tc.tile_pool(name="kvgp", bufs=3) as kvgp,
            tc.tile_pool(name="small", bufs=3) as small,
            tc.tile_pool(name="psA", bufs=2, space="PSUM") as psA,
            tc.tile_pool(name="psT", bufs=2, space="PSUM") as psT,
            tc.tile_pool(name="psF", bufs=2, space="PSUM") as psF,
        ):
            # ---------- constants ----------
            wq_sb = singles.tile([P, 2, HD], bf)
            nc.sync.dma_start(out=wq_sb[:], in_=wq[:])
            wkv_sb = singles.tile([P, 2, KVW], bf)
            nc.sync.dma_start(out=wkv_sb[:], in_=wkv[:])
            wo_sb = singles.tile([P, 2, D], bf)
            nc.sync.dma_start(out=wo_sb[:], in_=wo[:])
            bo_sb = singles.tile([1, D], bf)
            nc.sync.dma_start(out=bo_sb[:], in_=bo_p[:])
            nullkv_sb = singles.tile([P, KVW], bf)
            nc.sync.dma_start(out=nullkv_sb[:], in_=nullkv[:])
            ident_sb = singles.tile([P, P], bf)
            nc.sync.dma_start(out=ident_sb[:], in_=ident_p[:])
            idx_sb = singles.tile([P, idxcols], mybir.dt.int16)
            idx_dma = nc.sync.dma_start(out=idx_sb[:], in_=idxs_p[:])
            mask_sb = singles.tile([P, mcols], f32)
            nc.sync.dma_start(out=mask_sb[:], in_=masks_p[:])
            ones1 = singles.tile([1, P], bf)
            nc.vector.memset(ones1[:], 1.0)

            # ---------- phase A: kv table ----------
            x_sb = big.tile([P, 2, NKV], bf)
            nc.sync.dma_start(out=x_sb[:], in_=xT[:])
            kv_writes = []
            for i in range(KV_TILES):
                ps = psA.tile([P, KVW], f32, space="PSUM", tag="psA")
                nc.tensor.matmul(
                    out=ps[:], lhsT=x_sb[:, 0, i * P:(i + 1) * P],
                    rhs=wkv_sb[:, 0, :], start=True, stop=False)
                nc.tensor.matmul(
                    out=ps[:], lhsT=x_sb[:, 1, i * P:(i + 1) * P],
                    rhs=wkv_sb[:, 1, :], start=False, stop=True)
                st = kvstage.tile([P, KVW], bf, tag="kvstage")
                if i % 2 == 0:
                    nc.scalar.copy(out=st[:], in_=ps[:])
                else:
                    nc.vector.tensor_copy(out=st[:], in_=ps[:])
                kv_writes.append(
                    nc.sync.dma_start(out=kv_dram[i * P:(i + 1) * P, :], in_=st[:]))

            # ---------- phase B: q tiles ----------
            xp_sb = singles.tile([P, 2, NT * P], bf)
            nc.sync.dma_start(out=xp_sb[:], in_=xpT[:])
            q_sb = singles.tile([P, NT, HD], bf)
            for t in range(NT):
                psq = psF.tile([P, HD], f32, space="PSUM", tag="psF")
                nc.tensor.matmul(
                    out=psq[:], lhsT=xp_sb[:, 0, t * P:(t + 1) * P],
                    rhs=wq_sb[:, 0, :], start=True, stop=False)
                nc.tensor.matmul(
                    out=psq[:], lhsT=xp_sb[:, 1, t * P:(t + 1) * P],
                    rhs=wq_sb[:, 1, :], start=False, stop=True)
                nc.scalar.copy(out=q_sb[:, t, :], in_=psq[:])

            # ---------- phase C: attention per tile ----------
            if stage < 2:
                a_ts_eff = []
            else:
                a_ts_eff = a_ts
            io = 0
            mo = 0
            for t in range(len(a_ts_eff)):
                a = a_ts[t]
                aa = a + 1
                kv_g = kvgp.tile([P, aa, KVW], bf, tag="kvg")
                nc.scalar.copy(out=kv_g[:, 0, :], in_=nullkv_sb[:])
                c0 = 0
                while c0 < a:
                    c1 = min(c0 + 8, a)
                    gi = nc.gpsimd.dma_gather(
                        kv_g[:, 1 + c0:1 + c1, :], kv_dram[:],
                        idx_sb[:, io + 8 * c0:io + 8 * c1],
                        num_idxs=P * (c1 - c0), num_idxs_reg=P * (c1 - c0),
                        elem_size=KVW, single_packet=True,
                        queue_num=(t + (c0 // 8)) % 2)
                    # Tile's auto-dep tracking misses dma_gather's *input*
                    # APs (idx tile + DRAM source); add edges explicitly.
                    bass_rust.add_dep_helper(gi.ins, idx_dma.ins,
                                             reason="gather reads idx blob")
                    for kw in kv_writes:
                        bass_rust.add_dep_helper(gi.ins, kw.ins,
                                                 reason="gather reads kv table")
                    c0 = c1

                if stage < 3:
                    io += 8 * a
                    mo += aa
                    continue
                k4 = kv_g[:, :, 0:HD].rearrange("p a (h d) -> p a h d", d=DH)
                qb = (q_sb[:, t:t + 1, :]
                      .rearrange("p o (h d) -> p o h d", d=DH)
                      .broadcast_to([P, aa, H, DH]))
                nc.vector.tensor_tensor(out=k4, in0=k4, in1=qb, op=mult)
                w = DH
                while w > 1:
                    h2 = w // 2
                    nc.vector.tensor_tensor(
                        out=k4[:, :, :, 0:h2], in0=k4[:, :, :, 0:h2],
                        in1=k4[:, :, :, h2:w], op=add)
                    w = h2

                sim = kv_g[:, :, 0:HD:DH]          # [P, aa, H] strided
                exp_s = small.tile([P, aa, H], f32, tag="exp")
                nc.scalar.activation(
                    out=exp_s[:], in_=sim,
                    func=mybir.ActivationFunctionType.Exp)
                mb = (mask_sb[:, mo:mo + aa]
                      .rearrange("p (a o) -> p a o", o=1)
                      .broadcast_to([P, aa, H]))
                nc.vector.tensor_tensor(out=exp_s[:], in0=exp_s[:], in1=mb, op=mult)
                denom = small.tile([P, H], f32, tag="denom")
                nc.vector.tensor_reduce(
                    out=denom[:], in_=exp_s[:].rearrange("p a h -> p h a"),
                    axis=mybir.AxisListType.X, op=add)
                recip = small.tile([P, H], f32, tag="recip")
                nc.vector.reciprocal(out=recip[:], in_=denom[:])
                attn_b = small.tile([P, aa, H], f32, tag="attn")
                rb = (recip[:].rearrange("p (o h) -> p o h", o=1)
                      .broadcast_to([P, aa, H]))
                nc.vector.tensor_tensor(out=attn_b[:], in0=exp_s[:], in1=rb, op=mult)
                attn_x = work.tile([P, aa, H, DH], bf, tag="attnx")
                axin = (attn_b[:].rearrange("p a (h o) -> p a h o", o=1)
                        .broadcast_to([P, aa, H, DH]))
                nc.scalar.copy(out=attn_x[:], in_=axin)

                v4 = kv_g[:, :, HD:KVW].rearrange("p a (h d) -> p a h d", d=DH)
                nc.vector.tensor_tensor(out=v4, in0=v4, in1=attn_x[:], op=mult)
                w = aa
                while w > 1:
                    h2 = w // 2
                    nc.vector.tensor_tensor(
                        out=v4[:, 0:h2], in0=v4[:, 0:h2],
                        in1=v4[:, h2:2 * h2], op=add)
                    if w % 2 == 1:
                        nc.vector.tensor_tensor(
                            out=v4[:, 0:1], in0=v4[:, 0:1],
                            in1=v4[:, w - 1:w], op=add)
                    w = h2

                if stage < 4:
                    io += 8 * a
                    mo += aa
                    continue
                out_attn = kv_g[:, 0, HD:KVW]      # [P, 256] bf16
                outT = work.tile([P, 2, P], bf, tag="outT")
                for j in range(2):
                    pst = psT.tile([P, P], bf, space="PSUM", tag="psT")
                    nc.tensor.transpose(
                        out=pst[:], in_=out_attn[:, j * P:(j + 1) * P],
                        identity=ident_sb[:])
                    nc.scalar.copy(out=outT[:, j, :], in_=pst[:])

                psf = psF.tile([P, D], f32, space="PSUM", tag="psF")
                nc.tensor.matmul(out=psf[:], lhsT=ones1[0:1, :], rhs=bo_sb[0:1, :],
                                 start=True, stop=False)
                nc.tensor.matmul(out=psf[:], lhsT=outT[:, 0, :], rhs=wo_sb[:, 0, :],
                                 start=False, stop=False)
                nc.tensor.matmul(out=psf[:], lhsT=outT[:, 1, :], rhs=wo_sb[:, 1, :],
                                 start=False, stop=True)
                outf = small.tile([P, D], f32, tag="outf")
                nc.scalar.copy(out=outf[:], in_=psf[:])
                nc.sync.dma_start(out=out_p[t * P:(t + 1) * P, :], in_=outf[:])

                io += 8 * a
                mo += aa

    nc.finalize()
    return nc


def _prep(x, adj, msk, Wq, Wkv, Wo, bo, null_k, null_v):
    """All host-side numpy prep. Returns (a_ts, in_maps, order)."""
    deg = msk.sum(1).astype(np.int64)
    order = np.concatenate([
        np.full(NPAD - N, -1, dtype=np.int64),
        np.argsort(deg, kind="stable"),
    ])

    a_ts = []
    for t in range(NT):
        grp = order[t * GROUP:(t + 1) * GROUP]
        real = grp[grp >= 0]
        mx = int(deg[real].max()) if real.size else 0
        a_ts.append(max(mx, 1))

    # compact each node's neighbor list: valid entries first
    sortcols = np.argsort(~msk, axis=1, kind="stable")
    comp = np.take_along_axis(adj, sortcols, axis=1).astype(np.int16)

    scale = DH ** -0.5
    xpad = np.zeros((NKV, D), np.float32)
    xpad[:N] = x
    xT_h = np.ascontiguousarray(
        xpad.T.reshape(2, P, NKV).transpose(1, 0, 2)).astype(BF)
    wq_h = np.ascontiguousarray(
        (Wq * scale).reshape(2, P, HD).transpose(1, 0, 2)).astype(BF)
    wkv_h = np.ascontiguousarray(
        Wkv.reshape(2, P, KVW).transpose(1, 0, 2)).astype(BF)
    wo_h = np.ascontiguousarray(
        Wo.reshape(2, P, D).transpose(1, 0, 2)).astype(BF)
    bo_h = bo.reshape(1, D).astype(BF)
    nullkv_h = np.tile(
        np.concatenate([null_k.reshape(-1), null_v.reshape(-1)]).reshape(1, KVW),
        (P, 1)).astype(BF)
    ident_h = np.eye(P, dtype=np.float32).astype(BF)

    in_maps = []
    for c in range(NCORES):
        xp = np.zeros((NT * P, D), np.float32)
        flats = []
        mblocks = []
        for t in range(NT):
            a = a_ts[t]
            nodes = order[t * GROUP + c * P: t * GROUP + (c + 1) * P]
            nn = np.maximum(nodes, 0)
            xp[t * P:(t + 1) * P][nodes >= 0] = x[nodes[nodes >= 0]]
            valid = (np.arange(a)[None, :] < deg[nn][:, None]) & (nodes >= 0)[:, None]
            blk = np.where(valid, comp[nn, :a], 0).astype(np.int16)  # [128, a]
            flats.append(blk.T.reshape(-1))                          # i = col*128+p
            m = np.zeros((P, 1 + a), np.float32)
            m[:, 0] = 1.0
            m[:, 1:] = valid
            mblocks.append(m)
        flat = np.concatenate(flats)
        idx_h = np.ascontiguousarray(
            np.tile(flat.reshape(-1, 16).T, (8, 1))).astype(np.int16)
        mask_h = np.ascontiguousarray(np.concatenate(mblocks, axis=1))
        xpT_h = np.ascontiguousarray(
            xp.T.reshape(2, P, NT * P).transpose(1, 0, 2)).astype(BF)
        in_maps.append({
            "xT": xT_h, "xpT": xpT_h, "wq": wq_h, "wkv": wkv_h, "wo": wo_h,
            "bo": bo_h, "nullkv": nullkv_h, "ident": ident_h,
            "idxs": idx_h, "masks": mask_h,
        })
    return a_ts, in_maps, order


def kernel(x, adj_kv_indices, mask, Wq, Wkv, Wo, bo, null_k, null_v):
    global LAST_EXEC_NS
    x = np.asarray(x, dtype=np.float32)[0]
    adj = np.asarray(adj_kv_indices)[0].astype(np.int64)
    msk = np.asarray(mask)[0].astype(bool)
    Wq = np.asarray(Wq, np.float32)
    Wkv = np.asarray(Wkv, np.float32)
    Wo = np.asarray(Wo, np.float32)
    bo = np.asarray(bo, np.float32)
    null_k = np.asarray(null_k, np.float32)
    null_v = np.asarray(null_v, np.float32)

    a_ts, in_maps, order = _prep(x, adj, msk, Wq, Wkv, Wo, bo, null_k, null_v)
    nc = _build(tuple(a_ts))
    res = run_bass_kernel_spmd(
        nc, in_maps, core_ids=list(range(NCORES)),
        trace=bool(os.environ.get("KERNEL_TRACE")))
    LAST_EXEC_NS = res.exec_time_ns

    out_full = np.zeros((N, D), np.float32)
    for c in range(NCORES):
        o = np.asarray(res.results[c]["out"])
        for t in range(NT):
            nodes = order[t * GROUP + c * P: t * GROUP + (c + 1) * P]
            sel = nodes >= 0
            out_full[nodes[sel]] = o[t * P:(t + 1) * P][sel]
    return out_full.reshape(1, N, D)


# revision 7
# speedup vs baseline: 1.4025x; 1.1228x over previous
"""AdjacentAttention on 8 TRN2 NeuronCores.

Strategy (all shapes hardcoded for B=1, N=10000, A=32, D=256, H=4, DH=64):

Host:
  - kv projection commutes with the neighbor gather, so the device computes a
    kv table (x @ Wkv, bf16) once and gathers *projected* rows, 32x less
    matmul work than the reference's gather-then-project.
  - ~50% of neighbors are masked out.  The host compacts each node's
    neighbor list to its valid entries, sorts nodes by degree, and deals
    them into 10 degree-homogeneous tile groups of 1024 (128 nodes x 8
    cores), so each tile only gathers/computes its group-max degree a_t
    instead of A=32.
  - x is passed pre-transposed (and bf16) so no on-device transposes are
    needed for the matmuls; attention-scale is folded into Wq.

Device (SPMD, identical program per core, no collectives):
  Phase A: kv table = x @ Wkv for all 10112 padded rows -> HBM scratch (bf16).
  Phase B: q tiles for this core's 1280 permuted nodes.
  Phase C: per node-tile: dma_gather of kv rows for (node, valid-neighbor)
    pairs + a null slot; q.k on DVE (bf16 2x) + halving trees; softmax with
    f32 denominators; attn-weighted v-sum; PE transpose + out-projection.
"""

import os

import numpy as np
import ml_dtypes

import bass_rust
import concourse.bacc as bacc
import concourse.tile as tile
from concourse import mybir
from concourse.bass_utils import run_bass_kernel_spmd

BF = ml_dtypes.bfloat16

N, A, D, H, DH = 10000, 32, 256, 4, 64
NCORES, P, NT = 8, 128, 10
GROUP = NCORES * P            # 1024 nodes per tile-group
NPAD = NT * GROUP             # 10240
KV_TILES = (N + P - 1) // P   # 79
NKV = KV_TILES * P            # 10112 padded kv-table rows
HD = H * DH                   # 256
KVW = 2 * HD                  # 512 (k|v row width)

LAST_EXEC_NS = None

_MULT = None
_ADD = None


def _build(a_ts):
    stage = int(os.environ.get("KERNEL_STAGE", "4"))
    nc = bacc.Bacc("TRN2", target_bir_lowering=False, num_swdge_queues=2)
    bf = mybir.dt.bfloat16
    f32 = mybir.dt.float32
    mult = mybir.AluOpType.mult
    add = mybir.AluOpType.add

    aas = [a + 1 for a in a_ts]
    idxcols = 8 * sum(a_ts)
    mcols = sum(aas)

    xT = nc.declare_dram_parameter("xT", [P, 2, NKV], bf, isOutput=False)
    xpT = nc.declare_dram_parameter("xpT", [P, 2, NT * P], bf, isOutput=False)
    wq = nc.declare_dram_parameter("wq", [P, 2, HD], bf, isOutput=False)
    wkv = nc.declare_dram_parameter("wkv", [P, 2, KVW], bf, isOutput=False)
    wo = nc.declare_dram_parameter("wo", [P, 2, D], bf, isOutput=False)
    bo_p = nc.declare_dram_parameter("bo", [1, D], bf, isOutput=False)
    nullkv = nc.declare_dram_parameter("nullkv", [P, KVW], bf, isOutput=False)
    ident_p = nc.declare_dram_parameter("ident", [P, P], bf, isOutput=False)
    idxs_p = nc.declare_dram_parameter("idxs", [P, idxcols], mybir.dt.int16, isOutput=False)
    masks_p = nc.declare_dram_parameter("masks", [P, mcols], f32, isOutput=False)
    out_p = nc.declare_dram_parameter("out", [NT * P, D], f32, isOutput=True)

    kv_dram = nc.dram_tensor("kv_scratch", [NKV, KVW], bf)

    with tile.TileContext(nc) as tc:
        with (
            tc.tile_pool(name="big", bufs=1) as big,
            tc.tile_pool(name="singles", bufs=1) as singles,
            tc.tile_pool(name="kvstage", bufs=4) as kvstage,
            tc.tile_pool(name="work", bufs=2) as work,
            tc.tile_pool(name="kvgp", bufs=4) as kvgp,
            tc.tile_pool(name="small", bufs=3) as small,
            tc.tile_pool(name="psA", bufs=2, space="PSUM") as psA,
            tc.tile_pool(name="psT", bufs=2, space="PSUM") as psT,
            tc.tile_pool(name="psF", bufs=2, space="PSUM") as psF,
        ):
            # ---------- constants ----------
            wq_sb = singles.tile([P, 2, HD], bf)
            nc.sync.dma_start(out=wq_sb[:], in_=wq[:])
            wkv_sb = singles.tile([P, 2, KVW], bf)
            nc.sync.dma_start(out=wkv_sb[:], in_=wkv[:])
            wo_sb = singles.tile([P, 2, D], bf)
            nc.sync.dma_start(out=wo_sb[:], in_=wo[:])
            bo_sb = singles.tile([1, D], bf)
            nc.sync.dma_start(out=bo_sb[:], in_=bo_p[:])
            nullkv_sb = singles.tile([P, KVW], bf)
            nc.sync.dma_start(out=nullkv_sb[:], in_=nullkv[:])
            ident_sb = singles.tile([P, P], bf)
            nc.sync.dma_start(out=ident_sb[:], in_=ident_p[:])
            idx_sb = singles.tile([P, idxcols], mybir.dt.int16)
            idx_dma = nc.sync.dma_start(out=idx_sb[:], in_=idxs_p[:])
            mask_sb = singles.tile([P, mcols], f32)
            nc.sync.dma_start(out=mask_sb[:], in_=masks_p[:])
            ones1 = singles.tile([1, P], bf)
            nc.vector.memset(ones1[:], 1.0)

            # ---------- phase A: kv table ----------
            x_sb = big.tile([P, 2, NKV], bf)
            nc.sync.dma_start(out=x_sb[:], in_=xT[:])
            kv_writes = []
            for i in range(KV_TILES):
                ps = psA.tile([P, KVW], f32, space="PSUM", tag="psA")
                nc.tensor.matmul(
                    out=ps[:], lhsT=x_sb[:, 0, i * P:(i + 1) * P],
                    rhs=wkv_sb[:, 0, :], start=True, stop=False)
                nc.tensor.matmul(
                    out=ps[:], lhsT=x_sb[:, 1, i * P:(i + 1) * P],
                    rhs=wkv_sb[:, 1, :], start=False, stop=True)
                st = kvstage.tile([P, KVW], bf, tag="kvstage")
                if i % 2 == 0:
                    nc.scalar.copy(out=st[:], in_=ps[:])
                else:
                    nc.vector.tensor_copy(out=st[:], in_=ps[:])
                kv_writes.append(
                    nc.sync.dma_start(out=kv_dram[i * P:(i + 1) * P, :], in_=st[:]))

            # ---------- phase B: q tiles ----------
            xp_sb = singles.tile([P, 2, NT * P], bf)
            nc.sync.dma_start(out=xp_sb[:], in_=xpT[:])
            q_sb = singles.tile([P, NT, HD], bf)
            for t in range(NT):
                psq = psF.tile([P, HD], f32, space="PSUM", tag="psF")
                nc.tensor.matmul(
                    out=psq[:], lhsT=xp_sb[:, 0, t * P:(t + 1) * P],
                    rhs=wq_sb[:, 0, :], start=True, stop=False)
                nc.tensor.matmul(
                    out=psq[:], lhsT=xp_sb[:, 1, t * P:(t + 1) * P],
                    rhs=wq_sb[:, 1, :], start=False, stop=True)
                nc.scalar.copy(out=q_sb[:, t, :], in_=psq[:])

            # ---------- phase C: attention per tile ----------
            if stage < 2:
                a_ts_eff = []
            else:
                a_ts_eff = a_ts
            io = 0
            mo = 0
            for t in range(len(a_ts_eff)):
                a = a_ts[t]
                aa = a + 1
                kv_g = kvgp.tile([P, aa, KVW], bf, tag="kvg")
                nc.scalar.copy(out=kv_g[:, 0, :], in_=nullkv_sb[:])
                c0 = 0
                while c0 < a:
                    c1 = min(c0 + 8, a)
                    gi = nc.gpsimd.dma_gather(
                        kv_g[:, 1 + c0:1 + c1, :], kv_dram[:],
                        idx_sb[:, io + 8 * c0:io + 8 * c1],
                        num_idxs=P * (c1 - c0), num_idxs_reg=P * (c1 - c0),
                        elem_size=KVW, single_packet=False,
                        queue_num=(t + (c0 // 8)) % 2)
                    # Tile's auto-dep tracking misses dma_gather's *input*
                    # APs (idx tile + DRAM source); add edges explicitly.
                    bass_rust.add_dep_helper(gi.ins, idx_dma.ins,
                                             reason="gather reads idx blob")
                    for kw in kv_writes:
                        bass_rust.add_dep_helper(gi.ins, kw.ins,
                                                 reason="gather reads kv table")
                    c0 = c1

                if stage < 3:
                    io += 8 * a
                    mo += aa
                    continue
                k4 = kv_g[:, :, 0:HD].rearrange("p a (h d) -> p a h d", d=DH)
                qb = (q_sb[:, t:t + 1, :]
                      .rearrange("p o (h d) -> p o h d", d=DH)
                      .broadcast_to([P, aa, H, DH]))
                nc.vector.tensor_tensor(out=k4, in0=k4, in1=qb, op=mult)
                w = DH
                while w > 1:
                    h2 = w // 2
                    nc.vector.tensor_tensor(
                        out=k4[:, :, :, 0:h2], in0=k4[:, :, :, 0:h2],
                        in1=k4[:, :, :, h2:w], op=add)
                    w = h2

                sim = kv_g[:, :, 0:HD:DH]          # [P, aa, H] strided
                exp_s = small.tile([P, aa, H], f32, tag="exp")
                nc.scalar.activation(
                    out=exp_s[:], in_=sim,
                    func=mybir.ActivationFunctionType.Exp)
                mb = (mask_sb[:, mo:mo + aa]
                      .rearrange("p (a o) -> p a o", o=1)
                      .broadcast_to([P, aa, H]))
                nc.vector.tensor_tensor(out=exp_s[:], in0=exp_s[:], in1=mb, op=mult)
                denom = small.tile([P, H], f32, tag="denom")
                nc.vector.tensor_reduce(
                    out=denom[:], in_=exp_s[:].rearrange("p a h -> p h a"),
                    axis=mybir.AxisListType.X, op=add)
                recip = small.tile([P, H], f32, tag="recip")
                nc.vector.reciprocal(out=recip[:], in_=denom[:])
                attn_b = small.tile([P, aa, H], f32, tag="attn")
                rb = (recip[:].rearrange("p (o h) -> p o h", o=1)
                      .broadcast_to([P, aa, H]))
                nc.vector.tensor_tensor(out=attn_b[:], in0=exp_s[:], in1=rb, op=mult)
                attn_x = work.tile([P, aa, H, DH], bf, tag="attnx")
                axin = (attn_b[:].rearrange("p a (h o) -> p a h o", o=1)
                        .broadcast_to([P, aa, H, DH]))
                nc.scalar.copy(out=attn_x[:], in_=axin)

                v4 = kv_g[:, :, HD:KVW].rearrange("p a (h d) -> p a h d", d=DH)
                nc.vector.tensor_tensor(out=v4, in0=v4, in1=attn_x[:], op=mult)
                w = aa
                while w > 1:
                    h2 = w // 2
                    nc.vector.tensor_tensor(
                        out=v4[:, 0:h2], in0=v4[:, 0:h2],
                        in1=v4[:, h2:2 * h2], op=add)
                    if w % 2 == 1:
                        nc.vector.tensor_tensor(
                            out=v4[:, 0:1], in0=v4[:, 0:1],
                            in1=v4[:, w - 1:w], op=add)
                    w = h2

                if stage < 4:
                    io += 8 * a
                    mo += aa
                    continue
                out_attn = kv_g[:, 0, HD:KVW]      # [P, 256] bf16
                outT = work.tile([P, 2, P], bf, tag="outT")
                for j in range(2):
                    pst = psT.tile([P, P], bf, space="PSUM", tag="psT")
                    nc.tensor.transpose(
                        out=pst[:], in_=out_attn[:, j * P:(j + 1) * P],
                        identity=ident_sb[:])
                    nc.scalar.copy(out=outT[:, j, :], in_=pst[:])

                psf = psF.tile([P, D], f32, space="PSUM", tag="psF")
                nc.tensor.matmul(out=psf[:], lhsT=ones1[0:1, :], rhs=bo_sb[0:1, :],
                                 start=True, stop=False)
                nc.tensor.matmul(out=psf[:], lhsT=outT[:, 0, :], rhs=wo_sb[:, 0, :],
                                 start=False, stop=False)
                nc.tensor.matmul(out=psf[:], lhsT=outT[:, 1, :], rhs=wo_sb[:, 1, :],
                                 start=False, stop=True)
                outf = small.tile([P, D], f32, tag="outf")
                nc.scalar.copy(out=outf[:], in_=psf[:])
                nc.sync.dma_start(out=out_p[t * P:(t + 1) * P, :], in_=outf[:])

                io += 8 * a
                mo += aa

    nc.finalize()
    return nc


def _prep(x, adj, msk, Wq, Wkv, Wo, bo, null_k, null_v):
    """All host-side numpy prep. Returns (a_ts, in_maps, order)."""
    deg = msk.sum(1).astype(np.int64)
    order = np.concatenate([
        np.full(NPAD - N, -1, dtype=np.int64),
        np.argsort(deg, kind="stable"),
    ])

    a_ts = []
    for t in range(NT):
        grp = order[t * GROUP:(t + 1) * GROUP]
        real = grp[grp >= 0]
        mx = int(deg[real].max()) if real.size else 0
        a_ts.append(max(mx, 1))

    # compact each node's neighbor list: valid entries first
    sortcols = np.argsort(~msk, axis=1, kind="stable")
    comp = np.take_along_axis(adj, sortcols, axis=1).astype(np.int16)

    scale = DH ** -0.5
    xpad = np.zeros((NKV, D), np.float32)
    xpad[:N] = x
    xT_h = np.ascontiguousarray(
        xpad.T.reshape(2, P, NKV).transpose(1, 0, 2)).astype(BF)
    wq_h = np.ascontiguousarray(
        (Wq * scale).reshape(2, P, HD).transpose(1, 0, 2)).astype(BF)
    wkv_h = np.ascontiguousarray(
        Wkv.reshape(2, P, KVW).transpose(1, 0, 2)).astype(BF)
    wo_h = np.ascontiguousarray(
        Wo.reshape(2, P, D).transpose(1, 0, 2)).astype(BF)
    bo_h = bo.reshape(1, D).astype(BF)
    nullkv_h = np.tile(
        np.concatenate([null_k.reshape(-1), null_v.reshape(-1)]).reshape(1, KVW),
        (P, 1)).astype(BF)
    ident_h = np.eye(P, dtype=np.float32).astype(BF)

    in_maps = []
    for c in range(NCORES):
        xp = np.zeros((NT * P, D), np.float32)
        flats = []
        mblocks = []
        for t in range(NT):
            a = a_ts[t]
            nodes = order[t * GROUP + c * P: t * GROUP + (c + 1) * P]
            nn = np.maximum(nodes, 0)
            xp[t * P:(t + 1) * P][nodes >= 0] = x[nodes[nodes >= 0]]
            valid = (np.arange(a)[None, :] < deg[nn][:, None]) & (nodes >= 0)[:, None]
            blk = np.where(valid, comp[nn, :a], 0).astype(np.int16)  # [128, a]
            flats.append(blk.T.reshape(-1))                          # i = col*128+p
            m = np.zeros((P, 1 + a), np.float32)
            m[:, 0] = 1.0
            m[:, 1:] = valid
            mblocks.append(m)
        flat = np.concatenate(flats)
        idx_h = np.ascontiguousarray(
            np.tile(flat.reshape(-1, 16).T, (8, 1))).astype(np.int16)
        mask_h = np.ascontiguousarray(np.concatenate(mblocks, axis=1))
        xpT_h = np.ascontiguousarray(
            xp.T.reshape(2, P, NT * P).transpose(1, 0, 2)).astype(BF)
        in_maps.append({
            "xT": xT_h, "xpT": xpT_h, "wq": wq_h, "wkv": wkv_h, "wo": wo_h,
            "bo": bo_h, "nullkv": nullkv_h, "ident": ident_h,
            "idxs": idx_h, "masks": mask_h,
        })
    return a_ts, in_maps, order


def kernel(x, adj_kv_indices, mask, Wq, Wkv, Wo, bo, null_k, null_v):
    global LAST_EXEC_NS
    x = np.asarray(x, dtype=np.float32)[0]
    adj = np.asarray(adj_kv_indices)[0].astype(np.int64)
    msk = np.asarray(mask)[0].astype(bool)
    Wq = np.asarray(Wq, np.float32)
    Wkv = np.asarray(Wkv, np.float32)
    Wo = np.asarray(Wo, np.float32)
    bo = np.asarray(bo, np.float32)
    null_k = np.asarray(null_k, np.float32)
    null_v = np.asarray(null_v, np.float32)

    a_ts, in_maps, order = _prep(x, adj, msk, Wq, Wkv, Wo, bo, null_k, null_v)
    nc = _build(tuple(a_ts))
    res = run_bass_kernel_spmd(
        nc, in_maps, core_ids=list(range(NCORES)),
        trace=bool(os.environ.get("KERNEL_TRACE")))
    LAST_EXEC_NS = res.exec_time_ns

    out_full = np.zeros((N, D), np.float32)
    for c in range(NCORES):
        o = np.asarray(res.results[c]["out"])
        for t in range(NT):
            nodes = order[t * GROUP + c * P: t * GROUP + (c + 1) * P]
            sel = nodes >= 0
            out_full[nodes[sel]] = o[t * P:(t + 1) * P][sel]
    return out_full.reshape(1, N, D)
